# revision 1
# baseline (speedup 1.0000x reference)
"""Trainium2 Bass kernel for nn_Attention_84679575208344 (Performer-style
linear attention). Data-parallel over batch: 8 batches -> 8 NeuronCores.

Math per batch b (reference):
  qkv = x @ Wqkv.T -> split q,k,v per head (HD=48)
  qp = relu(dn*q)+1e-3 ; kp = relu(dn*k)+1e-3          (dn = 48**-0.25)
  ks = kp.sum(n) ; D = qp @ ks ; kptv = v.T @ kp (per head)
  attn = (qp @ kptv.T) / (D + 1e-8)
  out  = reshape(B,H,N,HD)->(B,N,C) WITHOUT head transpose, then @ Wproj.T + b

The no-transpose reshape means output row n' = 512*h + q holds
attn[h, 8q+j, d] at column 48j+d. We compute attention transposed
(features on partitions), build A^T[c''=64j+d, q] directly, and do the
projection with a head-padded Wproj^T (zero rows kill the padding).
"""

from contextlib import ExitStack

import numpy as np

import concourse.bass as bass
import concourse.mybir as mybir
import concourse.tile as tile
from concourse import bacc

F32 = mybir.dt.float32
F32R = mybir.dt.float32r
BF16 = mybir.dt.bfloat16
AL = mybir.AluOpType
FCOPY = mybir.ActivationFunctionType.Copy
FID = mybir.ActivationFunctionType.Identity

B, N, C, H = 8, 4096, 384, 8
HD = 48
KEPS = 1e-3
EPS = 1e-8
DN = float(HD ** (-0.25))
NCHUNK = N // 128  # 32
NBLK = N // 512    # 8

_NC_CACHE = {}


def _rep_row(src_ap, n):
    """Replicate a [1, F] SBUF row AP to n rows via a zero-step middle dim."""
    return bass.AP(tensor=src_ap.tensor, offset=src_ap.offset,
                   ap=[src_ap.ap[0], [0, n], src_ap.ap[1]])


def build_nc():
    nc = bacc.Bacc("TRN2", target_bir_lowering=False, debug=False, num_devices=8)
    x = nc.declare_dram_parameter("x", [N, C], F32, isOutput=False)
    wq = nc.declare_dram_parameter("wq", [C, 512], F32, isOutput=False)
    wkv = nc.declare_dram_parameter("wkv", [C, 768], F32, isOutput=False)
    wp = nc.declare_dram_parameter("wp", [512, C], F32, isOutput=False)
    bias = nc.declare_dram_parameter("bias", [C], F32, isOutput=False)
    ident_d = nc.declare_dram_parameter("ident", [128, 128], F32, isOutput=False)
    out = nc.declare_dram_parameter("out", [N, C], F32, isOutput=True)

    with tile.TileContext(nc) as tc, ExitStack() as ctx:
        persist = ctx.enter_context(tc.tile_pool(name="persist", bufs=1))
        xin_p = ctx.enter_context(tc.tile_pool(name="xin", bufs=3))
        kp_p = ctx.enter_context(tc.tile_pool(name="kp", bufs=2))
        v_p = ctx.enter_context(tc.tile_pool(name="v", bufs=2))
        rbig_p = ctx.enter_context(tc.tile_pool(name="rbig", bufs=4))
        ab_p = ctx.enter_context(tc.tile_pool(name="ab", bufs=2))
        zo_p = ctx.enter_context(tc.tile_pool(name="zo", bufs=3))

        qpT = persist.tile([128, 4, N], F32R)     # padded qp^T: head h at [64*(h%2)+d, h//2]
        wq_sb = persist.tile([128, 3, 512], F32R)
        wkv_sb = persist.tile([128, 3, 768], F32R)
        wp_sb = persist.tile([128, 4, C], F32R)
        ident = persist.tile([128, 128], F32)
        ones82 = persist.tile([128, 8, 2], BF16)
        kptv_sb = persist.tile([128, 4, 49], F32R)  # [m(+64 for odd h), h//2, d|ks]
        ks_f = persist.tile([128, 4, 8], F32)
        ks_sb = persist.tile([128, 4, 8], F32R)
        row_mask = persist.tile([128, 1], F32)

        nc.sync.dma_start(out=ident[:], in_=ident_d[:])
        nc.gpsimd.dma_start(out=wkv_sb[:], in_=wkv[:].rearrange("(c p) d -> p c d", p=128))
        nc.gpsimd.dma_start(out=wq_sb[:], in_=wq[:].rearrange("(c p) d -> p c d", p=128))
        nc.gpsimd.dma_start(out=wp_sb[:], in_=wp[:].rearrange("(c p) d -> p c d", p=128))
        nc.vector.memset(ones82[:], 1.0)
        nc.vector.memset(ks_f[:], 0.0)
        nc.vector.memset(row_mask[:], 0.0)
        one_f = persist.tile([1, 1], F32)
        nc.vector.memset(one_f[:], 1.0)
        ocell = one_f[0:1, 0:1]
        nc.sync.dma_start(out=row_mask[63:64, 0:1], in_=ocell)

        # ---------------- phase 1: x^T, K/V, kptv, qp^T ----------------
        with tc.tile_pool(name="ptrq", bufs=3, space="PSUM") as ptrq_p, \
             tc.tile_pool(name="pkv", bufs=2, space="PSUM") as pkv_p, \
             tc.tile_pool(name="pkp", bufs=1, space="PSUM") as pkp_p, \
             tc.tile_pool(name="xtp", bufs=1) as xt_p:
            psum_kptv = pkp_p.tile([48, 8, 50], F32)
            xT = xt_p.tile([128, 3, N], F32R)  # x^T; dies with phase 1

            def emit_q_block(blk):
                bs = slice(512 * blk, 512 * (blk + 1))
                for mc in range(4):
                    pq = ptrq_p.tile([128, 512], F32, tag="ptrq")
                    for kc in range(3):
                        nc.tensor.matmul(pq[:], wq_sb[:, kc, 128 * mc:128 * (mc + 1)],
                                         xT[:, kc, bs],
                                         start=(kc == 0), stop=(kc == 2))
                    nc.vector.tensor_scalar(qpT[:, mc, bs], pq[:], 0.0, KEPS,
                                            op0=AL.max, op1=AL.add)

            for i in range(NCHUNK):
                ns = slice(128 * i, 128 * (i + 1))
                xin = xin_p.tile([128, C], F32)
                nc.scalar.dma_start(out=xin[:], in_=x[ns, :])
                ptr = ptrq_p.tile([128, 512], F32, tag="ptrq")
                for kc in range(3):
                    nc.tensor.matmul(ptr[:, 128 * kc:128 * (kc + 1)],
                                     xin[:, 128 * kc:128 * (kc + 1)], ident[:],
                                     is_transpose=True, start=True, stop=True)
                for kc in range(3):
                    nc.scalar.copy(out=xT[:, kc, ns],
                                   in_=ptr[:, 128 * kc:128 * (kc + 1)])
                pkv = pkv_p.tile([128, 768], F32)
                for kc in range(3):
                    lhs = xT[:, kc, ns]
                    nc.tensor.matmul(pkv[:, 0:512], lhs, wkv_sb[:, kc, 0:512],
                                     start=(kc == 0), stop=(kc == 2))
                    nc.tensor.matmul(pkv[:, 512:768], lhs, wkv_sb[:, kc, 512:768],
                                     start=(kc == 0), stop=(kc == 2))
                kp = kp_p.tile([128, C], BF16)
                nc.vector.tensor_scalar(kp[:], pkv[:, 0:C], 0.0, KEPS,
                                        op0=AL.max, op1=AL.add)
                v = v_p.tile([128, 8, 50], BF16)
                nc.scalar.copy(
                    out=v[:, :, 0:48],
                    in_=pkv[:, C:768].rearrange("p (h d) -> p h d", h=8))
                nc.vector.tensor_copy(out=v[:, :, 48:50], in_=ones82[:])
                for h in range(H):
                    nc.tensor.matmul(psum_kptv[:, h, :], kp[:, 48 * h:48 * (h + 1)],
                                     v[:, h, :],
                                     start=(i == 0 and h == 0),
                                     stop=(i == NCHUNK - 1 and h == H - 1))
                if i % 4 == 0 and i > 0:
                    emit_q_block(i // 4 - 1)
            emit_q_block(NBLK - 1)

            # kptv psum -> sbuf, then DMA-remap heads to their qpT partition homes
            kptv_tmp = persist.tile([48, 4, 49], F32R)
            nc.vector.tensor_copy(out=kptv_sb[0:48, :, :],
                                  in_=psum_kptv[:, 0::2, 0:49])
            nc.vector.tensor_copy(out=kptv_tmp[:], in_=psum_kptv[:, 1::2, 0:49])
            nc.sync.dma_start(out=kptv_sb[64:112, :, :], in_=kptv_tmp[:])
            for h in range(H):
                p0 = 64 * (h % 2)
                nc.vector.tensor_copy(out=ks_f[p0:p0 + 48, h // 2, h:h + 1],
                                      in_=kptv_sb[p0:p0 + 48, h // 2, 48:49])
            nc.vector.tensor_copy(out=ks_sb[:], in_=ks_f[:])

        # ---------------- phase 2+3: D, attention, projection ----------------
        tc.strict_bb_all_engine_barrier()
        with tc.tile_pool(name="po", bufs=3, space="PSUM") as po_p, \
             tc.tile_pool(name="pd", bufs=2, space="PSUM") as pd_p, \
             tc.tile_pool(name="pz", bufs=3, space="PSUM") as pz_p, \
             tc.tile_pool(name="p23", bufs=1) as p23_p:
            rd_all = p23_p.tile([8, N], F32)
            at0 = p23_p.tile([128, 4, 512], F32R, tag="at0")
            at1 = p23_p.tile([128, 4, 512], F32R, tag="at1")
            zsrc = wkv_sb[:, :, :].rearrange("p a b -> p (a b)")
            for at in (at0, at1):
                nc.scalar.activation(
                    at[32:64, :, :].rearrange("p a b -> p (a b)"),
                    zsrc[32:64, 0:2048], FID,
                    bias=row_mask[32:64, :], scale=0.0)
                nc.scalar.activation(
                    at[96:128, :, :].rearrange("p a b -> p (a b)"),
                    zsrc[96:128, 0:2048], FCOPY, bias=0.0, scale=0.0)

            rdj = rd_all[:].rearrange("p (r j) -> p j r", j=8)
            # D matmuls interleaved with heads 0/1 on the unnormalized path:
            # their attention matmuls + explicit 1/D muls fill the PE pipeline
            # while D/recips for the remaining heads are still being computed.
            qh01 = [qpT[64 * hh:64 * hh + 48, 0, :].rearrange("p (r j) -> p j r", j=8)
                    for hh in range(2)]
            for j in range(8):
                pd = pd_p.tile([8, 512], F32)
                for cc in range(4):
                    rhs = qpT[:, cc, :].rearrange("p (r j) -> p j r", j=8)[:, j, :]
                    nc.tensor.matmul(pd[:], ks_sb[:, cc, :], rhs,
                                     start=(cc == 0), stop=(cc == 3))
                rcj = rbig_p.tile([8, 512], F32, tag="rcj")
                nc.vector.tensor_scalar_add(rcj[:], pd[:], EPS)
                nc.vector.reciprocal(rcj[:], rcj[:])
                nc.vector.tensor_copy(out=rdj[:, j, :], in_=rcj[:])
                for hh in range(2):
                    p0 = 64 * hh
                    at = at0 if hh == 0 else at1
                    po = po_p.tile([48, 512], F32)
                    nc.tensor.matmul(po[:], kptv_sb[p0:p0 + 48, 0, 0:48],
                                     qh01[hh][:, j, :], start=True, stop=True,
                                     tile_position=(p0, 0))
                    rb = rbig_p.tile([48, 512], F32, tag="rb")
                    deng = nc.sync if (j + hh) % 2 == 0 else nc.scalar
                    deng.dma_start(out=rb[:], in_=_rep_row(rcj[hh:hh + 1, :], 48))
                    if j % 2 == 0:
                        nc.vector.tensor_mul(at[0:48, j // 2, :], po[:], rb[:])
                    else:
                        ab = ab_p.tile([48, 512], F32R)
                        nc.vector.tensor_mul(ab[:], po[:], rb[:])
                        reng = (nc.gpsimd, nc.sync, nc.scalar, nc.gpsimd)[(j // 2) % 4]
                        reng.dma_start(out=at[64:112, j // 2, :], in_=ab[:])

            # normalize qp by 1/D in place (division-free attention matmuls)
            def norm_chunk(cc):
                for blk in range(NBLK):
                    bs = slice(512 * blk, 512 * (blk + 1))
                    rbig = rbig_p.tile([128, 512], F32)
                    nc.sync.dma_start(out=rbig[0:64, :],
                                      in_=_rep_row(rd_all[2 * cc:2 * cc + 1, bs], 64))
                    nc.scalar.dma_start(out=rbig[64:128, :],
                                        in_=_rep_row(rd_all[2 * cc + 1:2 * cc + 2, bs], 64))
                    nc.vector.tensor_mul(qpT[:, cc, bs], qpT[:, cc, bs], rbig[:])

            def emit_attn_head(h):
                p0 = 64 * (h % 2)
                at = at0 if h % 2 == 0 else at1
                qh = qpT[p0:p0 + 48, h // 2, :].rearrange("p (r j) -> p j r", j=8)
                for j in range(8):
                    po = po_p.tile([48, 512], F32)
                    nc.tensor.matmul(po[:], kptv_sb[p0:p0 + 48, h // 2, 0:48],
                                     qh[:, j, :], start=True, stop=True,
                                     tile_position=(p0, 0))
                    if j % 2 == 0:
                        nc.vector.tensor_copy(out=at[0:48, j // 2, :], in_=po[:])
                    else:
                        ab = ab_p.tile([48, 512], F32R)
                        nc.scalar.copy(out=ab[:], in_=po[:])
                        reng = (nc.gpsimd, nc.sync, nc.scalar, nc.gpsimd)[(j // 2) % 4]
                        reng.dma_start(out=at[64:112, j // 2, :], in_=ab[:])
                return at

            def emit_proj_head(h, at):
                for rc in range(4):
                    pz = pz_p.tile([128, C], F32)
                    for cc in range(4):
                        nc.tensor.matmul(pz[:], at[:, cc, 128 * rc:128 * (rc + 1)],
                                         wp_sb[:, cc, :],
                                         start=(cc == 0), stop=(cc == 3))
                    zo = zo_p.tile([128, C], F32)
                    if rc % 2 == 0:
                        nc.vector.tensor_copy(out=zo[:], in_=pz[:])
                    else:
                        nc.scalar.copy(out=zo[:], in_=pz[:])
                    r0 = 512 * h + 128 * rc
                    nc.sync.dma_start(out=out[r0:r0 + 128, :], in_=zo[:])

            ats = {0: at0, 1: at1}
            for cc in range(1, 4):
                norm_chunk(cc)
                for h in (2 * cc, 2 * cc + 1):
                    emit_proj_head(h - 2, ats.pop(h - 2))
                    ats[h] = emit_attn_head(h)
            emit_proj_head(6, ats.pop(6))
            emit_proj_head(7, ats.pop(7))
    nc.finalize()
    return nc


def _prep_weights(Wqkv, Wproj, bproj=None):
    """Host-side weight prep: fold dn, pad head dims, build transposed layouts."""
    Wq = Wqkv[0:C, :]
    Wk = Wqkv[C:2 * C, :]
    Wv = Wqkv[2 * C:3 * C, :]
    wq = np.zeros((C, 512), np.float32)
    for h in range(H):
        wq[:, 64 * h:64 * h + 48] = (DN * Wq[48 * h:48 * (h + 1), :]).T
    wkv = np.concatenate([(DN * Wk).T, Wv.T], axis=1).astype(np.float32)
    wp = np.zeros((512, C), np.float32)
    WprojT = Wproj.T
    for j in range(8):
        wp[64 * j:64 * j + 48, :] = WprojT[48 * j:48 * (j + 1), :]
    if bproj is not None:
        wp[63, :] = bproj
    return wq, wkv, wp


def _run(inputs, trace=False):
    from concourse.bass_utils import run_bass_kernel_spmd

    x = np.ascontiguousarray(np.asarray(inputs["x"], dtype=np.float32))
    Wqkv = np.asarray(inputs["Wqkv"], dtype=np.float32)
    Wproj = np.asarray(inputs["Wproj"], dtype=np.float32)
    bproj = np.ascontiguousarray(np.asarray(inputs["bproj"], dtype=np.float32))
    wq, wkv, wp = _prep_weights(Wqkv, Wproj, bproj)

    if "nc" not in _NC_CACHE:
        _NC_CACHE["nc"] = build_nc()
    nc = _NC_CACHE["nc"]

    ident = np.eye(128, dtype=np.float32)
    in_maps = [
        {"x": np.ascontiguousarray(x[b]), "wq": wq, "wkv": wkv, "wp": wp,
         "bias": bproj, "ident": ident}
        for b in range(B)
    ]
    res = run_bass_kernel_spmd(nc, in_maps, list(range(8)), trace=trace)
    out = np.stack([res.results[b]["out"] for b in range(B)], axis=0)
    return out, res


def kernel(**inputs) -> np.ndarray:
    out, _ = _run(inputs, trace=False)
    return out


def kernel_profiled(**inputs):
    out, res = _run(inputs, trace=True)
    return out, res



# revision 29
# speedup vs baseline: 1.2084x; 1.2084x over previous
"""Trainium2 Bass kernel for nn_Attention_84679575208344 (Performer-style
linear attention). Data-parallel over batch: 8 batches -> 8 NeuronCores.

Math per batch b (reference):
  qkv = x @ Wqkv.T -> split q,k,v per head (HD=48)
  qp = relu(dn*q)+1e-3 ; kp = relu(dn*k)+1e-3          (dn = 48**-0.25)
  ks = kp.sum(n) ; D = qp @ ks ; kptv = v.T @ kp (per head)
  attn = (qp @ kptv.T) / (D + 1e-8)
  out  = reshape(B,H,N,HD)->(B,N,C) WITHOUT head transpose, then @ Wproj.T + b

Layout strategy:
  - x is transposed on the host and uploaded bf16 as xT [128, 3, N]
    (c = kc*128 + p), so no on-device transpose is needed.
  - All moving matmul operands are bf16 (1 cycle/row in all cases).
  - Heads live in 64-aligned partition quadrants: head h sits at partitions
    64*(h%2)..+48 of slab h//2.  Attention j-even/j-odd outputs go to PSUM
    partitions 0:48 / 64:112 of one bank via tile_position, so one copy
    assembles a j-pair.
  - ks rides as an extra column of v through the kptv matmul (output
    partitions are free).  D = ks . qp via small packed matmuls; 1/D is
    replicated across partitions with 8 large SBUF DMAs (bf16) and qp is
    pre-normalized in place (bf16 2x DVE mode).
  - The projection reads head-padded at tiles [128, 4, 512]; pad rows hit
    zero rows of wp, row 127 is constant 1.0 and row 127 of wp slab 3 holds
    the bias.
"""

from contextlib import ExitStack

import numpy as np
import ml_dtypes

import concourse.bass as bass
import concourse.mybir as mybir
import concourse.tile as tile
from concourse import bacc

F32 = mybir.dt.float32
F32R = mybir.dt.float32r
BF16 = mybir.dt.bfloat16
AL = mybir.AluOpType

B, N, C, H = 8, 4096, 384, 8
HD = 48
KEPS = 1e-3
DN = float(HD ** (-0.25))
NCHUNK = N // 128  # 32
NBLK = N // 512    # 8

_NC_CACHE = {}


def _copy(eng, out, in_):
    if hasattr(eng, "copy"):
        eng.copy(out=out, in_=in_)
    else:
        eng.tensor_copy(out=out, in_=in_)


def _rep_row(src_ap, n):
    """Replicate a [1, F] SBUF row AP to n rows via a zero-step middle dim."""
    return bass.AP(tensor=src_ap.tensor, offset=src_ap.offset,
                   ap=[src_ap.ap[0], [0, n], src_ap.ap[1]])


def build_nc():
    nc = bacc.Bacc("TRN2", target_bir_lowering=False, debug=False, num_devices=8)
    xt = nc.declare_dram_parameter("xt", [128, 3, N], BF16, isOutput=False)
    wq = nc.declare_dram_parameter("wq", [128, 3, 512], BF16, isOutput=False)
    wkv = nc.declare_dram_parameter("wkv", [128, 3, 768], BF16, isOutput=False)
    wp = nc.declare_dram_parameter("wp", [128, 4, C], BF16, isOutput=False)
    out = nc.declare_dram_parameter("out", [N, C], F32, isOutput=True)

    with tile.TileContext(nc) as tc, ExitStack() as ctx:
        persist = ctx.enter_context(tc.tile_pool(name="persist", bufs=1))
        kp_p = ctx.enter_context(tc.tile_pool(name="kp", bufs=2))
        v_p = ctx.enter_context(tc.tile_pool(name="v", bufs=3))
        zo_p = ctx.enter_context(tc.tile_pool(name="zo", bufs=3))

        xT = persist.tile([128, 3, N], BF16)
        wq_sb = persist.tile([128, 3, 512], BF16)
        wkv_sb = persist.tile([128, 3, 768], BF16)
        wp_sb = persist.tile([128, 4, C], BF16)
        qpT = persist.tile([128, 4, N], BF16)    # head h at [64*(h%2)+d, h//2]
        kptv_sb = persist.tile([128, 4, 49], BF16)  # col 48 = ks
        ks2 = persist.tile([128, 4, 8], BF16)    # ks: slab cc col h, zero-masked
        rd = persist.tile([8, N], BF16)          # 1/D per head (row h)
        rbig = persist.tile([128, 4, N], BF16)   # 1/D replicated to quadrants
        at0 = persist.tile([128, 4, 512], BF16, tag="at0")
        at1 = persist.tile([128, 4, 512], BF16, tag="at1")

        # weight + input loads (xt in 8 pipelined block chunks)
        nc.sync.dma_start(out=wq_sb[:], in_=wq[:])
        nc.scalar.dma_start(out=wkv_sb[:], in_=wkv[:])
        nc.gpsimd.dma_start(out=wp_sb[:], in_=wp[:])
        xt_eng = (nc.sync, nc.scalar, nc.gpsimd)
        for b in range(NBLK):
            bs = slice(512 * b, 512 * (b + 1))
            xt_eng[b % 3].dma_start(out=xT[:, :, bs], in_=xt[:, :, bs])

        nc.vector.memset(ks2[:], 0.0)
        # at pad rows: 48:64 and 112:127 must be finite (they hit zero wp
        # rows); row 127 is the bias row: 1.0 on slab 3, 0 elsewhere.
        # Engine writes need 32-aligned partition bases; row 127 goes via DMA.
        one_row = persist.tile([1, 512], BF16)
        nc.vector.memset(one_row[:], 1.0)
        for at in (at0, at1):
            nc.gpsimd.memset(at[32:64, :, :], 0.0)
            nc.gpsimd.memset(at[96:128, :, :], 0.0)
        nc.sync.dma_start(out=at0[127:128, 3, :], in_=one_row[:])
        nc.scalar.dma_start(out=at1[127:128, 3, :], in_=one_row[:])

        # ---------------- phase 1: K/V, kptv(+ks), qp^T ----------------
        with tc.tile_pool(name="pq", bufs=3, space="PSUM") as pq_p, \
             tc.tile_pool(name="pkv", bufs=2, space="PSUM") as pkv_p, \
             tc.tile_pool(name="pkp", bufs=1, space="PSUM") as pkp_p:
            # CoreSim's psum-group model only tracks partition-base-0 APs (or
            # exact [128,512] single-bank tiles) coherently, so kptv
            # accumulates at base 0 with heads along the free dim and is
            # remapped to quadrant homes afterwards.
            psum_kptv = pkp_p.tile([48, 8, 49], F32)

            def emit_q_block(blk):
                bs = slice(512 * blk, 512 * (blk + 1))
                for mc in range(4):
                    pq = pq_p.tile([128, 512], F32, tag="pq")
                    for kc in range(3):
                        nc.tensor.matmul(pq[:], wq_sb[:, kc, 128 * mc:128 * (mc + 1)],
                                         xT[:, kc, bs],
                                         start=(kc == 0), stop=(kc == 2))
                    nc.vector.tensor_scalar(qpT[:, mc, bs], pq[:], 0.0, KEPS,
                                      op0=AL.max, op1=AL.add)

            for i in range(NCHUNK):
                ns = slice(128 * i, 128 * (i + 1))
                pkv = pkv_p.tile([128, 768], F32)
                for kc in range(3):
                    lhs = xT[:, kc, ns]
                    nc.tensor.matmul(pkv[:, 0:512], lhs, wkv_sb[:, kc, 0:512],
                                     start=(kc == 0), stop=(kc == 2))
                    nc.tensor.matmul(pkv[:, 512:768], lhs, wkv_sb[:, kc, 512:768],
                                     start=(kc == 0), stop=(kc == 2))
                kp = kp_p.tile([128, C], BF16)
                nc.vector.tensor_scalar(kp[:], pkv[:, 0:C], 0.0, KEPS,
                                   op0=AL.max, op1=AL.add)
                v = v_p.tile([128, 8, 49], BF16, tag="v")
                veng = (nc.scalar, nc.vector)[i % 2]
                _copy(veng, v[:, :, 0:48],
                      pkv[:, C:768].rearrange("p (h d) -> p h d", h=8))
                nc.gpsimd.memset(v[:, :, 48:49], 1.0)
                for h in range(H):
                    nc.tensor.matmul(psum_kptv[:, h, :],
                                     kp[:, 48 * h:48 * (h + 1)], v[:, h, :],
                                     start=(i == 0 and h == 0),
                                     stop=(i == NCHUNK - 1 and h == H - 1))
                if i % 4 == 3:
                    emit_q_block(i // 4)

            # kptv psum -> quadrant homes: even heads copy in place, odd heads
            # hop partitions via a small SBUF-to-SBUF DMA.  ks rides col 48.
            tmp_o = kp_p.tile([48, 4, 49], BF16)
            nc.scalar.copy(out=kptv_sb[0:48, :, :], in_=psum_kptv[:, 0::2, :])
            nc.vector.tensor_copy(out=tmp_o[:], in_=psum_kptv[:, 1::2, :])
            nc.sync.dma_start(out=kptv_sb[64:112, :, :], in_=tmp_o[:])
            for h in range(H):
                q0 = 64 * (h % 2)
                nc.vector.tensor_copy(out=ks2[q0:q0 + 48, h // 2, h:h + 1],
                                      in_=kptv_sb[q0:q0 + 48, h // 2, 48:49])

        # ---------------- phase 2: D, attention, projection ----------------
        tc.strict_bb_all_engine_barrier()
        with tc.tile_pool(name="pd", bufs=2, space="PSUM") as pd_p, \
             tc.tile_pool(name="pa", bufs=4, space="PSUM") as pa_p, \
             tc.tile_pool(name="pz", bufs=2, space="PSUM") as pz_p:
            rdj = rd[:].rearrange("p (r j) -> p j r", j=8)
            qpj = [qpT[:, cc, :].rearrange("p (r j) -> p j r", j=8)
                   for cc in range(4)]
            for j in range(8):
                pd = pd_p.tile([8, 512], F32)
                for cc in range(4):
                    nc.tensor.matmul(pd[:], ks2[:, cc, :], qpj[cc][:, j, :],
                                     start=(cc == 0), stop=(cc == 3))
                with nc.allow_low_precision(reason="1/D in bf16 is plenty"):
                    nc.vector.reciprocal(rdj[:, j, :], pd[:])

            # replicate 1/D to the quadrant homes (bf16, one DMA per head)
            for h in range(H):
                q0 = 64 * (h % 2)
                eng = (nc.sync, nc.scalar, nc.gpsimd)[h % 3]
                eng.dma_start(out=rbig[q0:q0 + 64, h // 2, :],
                              in_=_rep_row(rd[h:h + 1, :], 64))

            # pre-normalize qp by 1/D in place (bf16: 2x DVE throughput)
            for cc in range(4):
                nc.vector.tensor_mul(qpT[:, cc, :], qpT[:, cc, :],
                                     rbig[:, cc, :])

            def emit_attn_head(h):
                cc, q0 = h // 2, 64 * (h % 2)
                at = at0 if h % 2 == 0 else at1
                kv = kptv_sb[q0:q0 + 48, cc, 0:48]
                qh = qpT[q0:q0 + 48, cc, :].rearrange("p (r j) -> p j r", j=8)
                for jp in range(4):
                    pa = pa_p.tile([128, 512], F32, tag="pa")
                    nc.tensor.matmul(pa[0:48, :], kv, qh[:, 2 * jp, :],
                                     start=True, stop=True,
                                     tile_position=(q0, 0))
                    nc.tensor.matmul(pa[64:112, :], kv, qh[:, 2 * jp + 1, :],
                                     start=True, stop=True,
                                     tile_position=(q0, 64))
                    _copy(nc.scalar, at[0:48, jp, :], pa[0:48, :])
                    _copy(nc.vector, at[64:112, jp, :], pa[64:112, :])
                return at

            def emit_proj_head(h, at):
                for rc in range(4):
                    pz = pz_p.tile([128, C], F32)
                    for cc in range(4):
                        nc.tensor.matmul(pz[:], at[:, cc, 128 * rc:128 * (rc + 1)],
                                         wp_sb[:, cc, :],
                                         start=(cc == 0), stop=(cc == 3))
                    zo = zo_p.tile([128, C], F32)
                    zeng = nc.scalar if rc % 2 == 0 else nc.vector
                    _copy(zeng, zo[:], pz[:])
                    r0 = 512 * h + 128 * rc
                    nc.sync.dma_start(out=out[r0:r0 + 128, :], in_=zo[:])

            prev = emit_attn_head(0)
            for h in range(1, H):
                cur = emit_attn_head(h)
                emit_proj_head(h - 1, prev)
                prev = cur
            emit_proj_head(H - 1, prev)
    nc.finalize()
    return nc


def _prep_weights(Wqkv, Wproj, bproj):
    """Host-side weight prep: fold dn, pad head dims, build device layouts."""
    Wq = Wqkv[0:C, :]
    Wk = Wqkv[C:2 * C, :]
    Wv = Wqkv[2 * C:3 * C, :]
    wq = np.zeros((C, 512), np.float32)
    for h in range(H):
        wq[:, 64 * h:64 * h + 48] = (DN * Wq[48 * h:48 * (h + 1), :]).T
    wq = np.ascontiguousarray(
        wq.reshape(3, 128, 512)).astype(ml_dtypes.bfloat16)
    wkv = np.concatenate([(DN * Wk).T, Wv.T], axis=1).astype(np.float32)
    wkv = np.ascontiguousarray(
        wkv.reshape(3, 128, 768)).astype(ml_dtypes.bfloat16)
    # wp [128, 4, C]: row p<48 -> slot j=2*cc; row 64<=p<112 -> slot j=2*cc+1
    wp = np.zeros((128, 4, C), np.float32)
    WprojT = np.ascontiguousarray(Wproj.T)
    for cc in range(4):
        wp[0:48, cc, :] = WprojT[48 * (2 * cc):48 * (2 * cc) + 48, :]
        wp[64:112, cc, :] = WprojT[48 * (2 * cc + 1):48 * (2 * cc + 1) + 48, :]
    wp[127, 3, :] = bproj
    return (np.ascontiguousarray(wq.transpose(1, 0, 2)),
            np.ascontiguousarray(wkv.transpose(1, 0, 2)),
            wp.astype(ml_dtypes.bfloat16))


def _prep_x(xb):
    """x [N, C] f32 -> xT [128, 3, N] bf16 (c = kc*128 + p)."""
    xt = np.ascontiguousarray(
        xb.T.reshape(3, 128, N).transpose(1, 0, 2)).astype(ml_dtypes.bfloat16)
    return xt


def _run(inputs, trace=False):
    from concourse.bass_utils import run_bass_kernel_spmd

    x = np.asarray(inputs["x"], dtype=np.float32)
    Wqkv = np.asarray(inputs["Wqkv"], dtype=np.float32)
    Wproj = np.asarray(inputs["Wproj"], dtype=np.float32)
    bproj = np.asarray(inputs["bproj"], dtype=np.float32)
    wq, wkv, wp = _prep_weights(Wqkv, Wproj, bproj)

    if "nc" not in _NC_CACHE:
        _NC_CACHE["nc"] = build_nc()
    nc = _NC_CACHE["nc"]

    in_maps = [
        {"xt": _prep_x(x[b]), "wq": wq, "wkv": wkv, "wp": wp}
        for b in range(B)
    ]
    res = run_bass_kernel_spmd(nc, in_maps, list(range(8)), trace=trace)
    out = np.stack([res.results[b]["out"] for b in range(B)], axis=0)
    return out, res


def kernel(**inputs) -> np.ndarray:
    out, _ = _run(inputs, trace=False)
    return out


def kernel_profiled(**inputs):
    out, res = _run(inputs, trace=True)
    return out, res


# revision 38
# speedup vs baseline: 1.3038x; 1.0790x over previous
"""Trainium2 Bass kernel for nn_Attention_84679575208344 (Performer-style
linear attention). Data-parallel over batch: 8 batches -> 8 NeuronCores.

Math per batch b (reference):
  qkv = x @ Wqkv.T -> split q,k,v per head (HD=48)
  qp = relu(dn*q)+1e-3 ; kp = relu(dn*k)+1e-3          (dn = 48**-0.25)
  ks = kp.sum(n) ; D = qp @ ks ; kptv = v.T @ kp (per head)
  attn = (qp @ kptv.T) / (D + 1e-8)
  out  = reshape(B,H,N,HD)->(B,N,C) WITHOUT head transpose, then @ Wproj.T + b

Layout strategy:
  - x is transposed on the host and uploaded bf16 as xT [128, 3, N]
    (c = kc*128 + p), so no on-device transpose is needed.
  - All moving matmul operands are bf16 (1 cycle/row in all cases).
  - Heads live in 64-aligned partition quadrants: head h sits at partitions
    64*(h%2)..+48 of slab h//2.  Attention j-even/j-odd outputs go to PSUM
    partitions 0:48 / 64:112 of one bank via tile_position, so one copy
    assembles a j-pair.
  - ks rides as an extra column of v through the kptv matmul (output
    partitions are free).  D = ks . qp via small packed matmuls; 1/D is
    replicated across partitions with 8 large SBUF DMAs (bf16) and qp is
    pre-normalized in place (bf16 2x DVE mode).
  - The projection reads head-padded at tiles [128, 4, 512]; pad rows hit
    zero rows of wp, row 127 is constant 1.0 and row 127 of wp slab 3 holds
    the bias.
"""

from contextlib import ExitStack

import numpy as np
import ml_dtypes

import concourse.bass as bass
import concourse.mybir as mybir
import concourse.tile as tile
from concourse import bacc

F32 = mybir.dt.float32
F32R = mybir.dt.float32r
BF16 = mybir.dt.bfloat16
AL = mybir.AluOpType

B, N, C, H = 8, 4096, 384, 8
HD = 48
KEPS = 1e-3
DN = float(HD ** (-0.25))
NCHUNK = N // 128  # 32
NBLK = N // 512    # 8

_NC_CACHE = {}


def _copy(eng, out, in_):
    if hasattr(eng, "copy"):
        eng.copy(out=out, in_=in_)
    else:
        eng.tensor_copy(out=out, in_=in_)


def _rep_row(src_ap, n):
    """Replicate a [1, F] SBUF row AP to n rows via a zero-step middle dim."""
    return bass.AP(tensor=src_ap.tensor, offset=src_ap.offset,
                   ap=[src_ap.ap[0], [0, n], src_ap.ap[1]])


def build_nc():
    nc = bacc.Bacc("TRN2", target_bir_lowering=False, debug=False, num_devices=8)
    xt = nc.declare_dram_parameter("xt", [128, 3, N], BF16, isOutput=False)
    wq = nc.declare_dram_parameter("wq", [128, 3, 512], BF16, isOutput=False)
    wkv = nc.declare_dram_parameter("wkv", [128, 3, 768], BF16, isOutput=False)
    wp = nc.declare_dram_parameter("wp", [128, 4, C], BF16, isOutput=False)
    out = nc.declare_dram_parameter("out", [N, C], F32, isOutput=True)

    with tile.TileContext(nc) as tc, ExitStack() as ctx:
        persist = ctx.enter_context(tc.tile_pool(name="persist", bufs=1))
        kp_p = ctx.enter_context(tc.tile_pool(name="kp", bufs=2))
        v_p = ctx.enter_context(tc.tile_pool(name="v", bufs=3))
        zo_p = ctx.enter_context(tc.tile_pool(name="zo", bufs=3))

        xT = persist.tile([128, 3, N], BF16)
        wq_sb = persist.tile([128, 3, 512], BF16)
        wkv_sb = persist.tile([128, 3, 768], BF16)
        wp_sb = persist.tile([128, 4, C], BF16)
        qpT = persist.tile([128, 4, N], BF16)    # head h at [64*(h%2)+d, h//2]
        kptv_sb = persist.tile([128, 4, 49], BF16)  # col 48 = ks
        ks2 = persist.tile([128, 4, 8], BF16)    # ks: slab cc col h, zero-masked
        rd = persist.tile([8, N], BF16)          # 1/D per head (row h)
        rbig = persist.tile([128, 4, N], BF16)   # 1/D replicated to quadrants
        at0 = persist.tile([128, 4, 512], BF16, tag="at0")
        at1 = persist.tile([128, 4, 512], BF16, tag="at1")

        # weight + input loads; wkv and xt block 0 gate the first matmul, so
        # they go first on separate queues
        nc.sync.dma_start(out=wkv_sb[:], in_=wkv[:])
        nc.scalar.dma_start(out=xT[:, :, 0:512], in_=xt[:, :, 0:512])
        nc.gpsimd.dma_start(out=wq_sb[:], in_=wq[:])
        xt_eng = (nc.sync, nc.scalar, nc.gpsimd)
        for b in range(1, NBLK):
            bs = slice(512 * b, 512 * (b + 1))
            xt_eng[b % 3].dma_start(out=xT[:, :, bs], in_=xt[:, :, bs])
        nc.sync.dma_start(out=wp_sb[:], in_=wp[:])

        nc.vector.memset(ks2[:], 0.0)
        # at pad rows: 48:64 and 112:127 must be finite (they hit zero wp
        # rows); row 127 is the bias row: 1.0 on slab 3, 0 elsewhere.
        # Engine writes need 32-aligned partition bases; row 127 goes via DMA.
        one_row = persist.tile([1, 512], BF16)
        nc.vector.memset(one_row[:], 1.0)
        for at in (at0, at1):
            nc.gpsimd.memset(at[32:64, :, :], 0.0)
            nc.gpsimd.memset(at[96:128, :, :], 0.0)
        nc.sync.dma_start(out=at0[127:128, 3, :], in_=one_row[:])
        nc.scalar.dma_start(out=at1[127:128, 3, :], in_=one_row[:])

        # ---------------- phase 1: K/V, kptv(+ks), qp^T ----------------
        with tc.tile_pool(name="pq", bufs=3, space="PSUM") as pq_p, \
             tc.tile_pool(name="pkv", bufs=2, space="PSUM") as pkv_p, \
             tc.tile_pool(name="pkp", bufs=1, space="PSUM") as pkp_p:
            # CoreSim's psum-group model only tracks partition-base-0 APs (or
            # exact [128,512] single-bank tiles) coherently, so kptv
            # accumulates at base 0 with heads along the free dim and is
            # remapped to quadrant homes afterwards.
            psum_kptv = pkp_p.tile([48, 8, 49], F32)

            def emit_q_block(blk):
                bs = slice(512 * blk, 512 * (blk + 1))
                for mc in range(4):
                    pq = pq_p.tile([128, 512], F32, tag="pq")
                    for kc in range(3):
                        nc.tensor.matmul(pq[:], wq_sb[:, kc, 128 * mc:128 * (mc + 1)],
                                         xT[:, kc, bs],
                                         start=(kc == 0), stop=(kc == 2))
                    nc.vector.tensor_scalar(qpT[:, mc, bs], pq[:], 0.0, KEPS,
                                      op0=AL.max, op1=AL.add)

            for i in range(NCHUNK):
                ns = slice(128 * i, 128 * (i + 1))
                pkv = pkv_p.tile([128, 768], F32)
                for kc in range(3):
                    lhs = xT[:, kc, ns]
                    nc.tensor.matmul(pkv[:, 0:512], lhs, wkv_sb[:, kc, 0:512],
                                     start=(kc == 0), stop=(kc == 2))
                    nc.tensor.matmul(pkv[:, 512:768], lhs, wkv_sb[:, kc, 512:768],
                                     start=(kc == 0), stop=(kc == 2))
                kp = kp_p.tile([128, C], BF16)
                nc.vector.tensor_scalar(kp[:], pkv[:, 0:C], 0.0, KEPS,
                                   op0=AL.max, op1=AL.add)
                v = v_p.tile([128, 8, 49], BF16, tag="v")
                veng = (nc.scalar, nc.scalar, nc.vector)[i % 3]
                _copy(veng, v[:, :, 0:48],
                      pkv[:, C:768].rearrange("p (h d) -> p h d", h=8))
                nc.gpsimd.memset(v[:, :, 48:49], 1.0)
                for h in range(H):
                    nc.tensor.matmul(psum_kptv[:, h, :],
                                     kp[:, 48 * h:48 * (h + 1)], v[:, h, :],
                                     start=(i == 0 and h == 0),
                                     stop=(i == NCHUNK - 1 and h == H - 1))
                if i % 4 == 3:
                    emit_q_block(i // 4)

            # kptv psum -> quadrant homes: even heads copy in place, odd heads
            # hop partitions via a small SBUF-to-SBUF DMA.  ks rides col 48.
            tmp_o = kp_p.tile([48, 4, 49], BF16)
            nc.scalar.copy(out=kptv_sb[0:48, :, :], in_=psum_kptv[:, 0::2, :])
            nc.vector.tensor_copy(out=tmp_o[:], in_=psum_kptv[:, 1::2, :])
            nc.sync.dma_start(out=kptv_sb[64:112, :, :], in_=tmp_o[:])
            for cc in range(4):
                nc.vector.tensor_copy(out=ks2[0:48, cc, 2 * cc:2 * cc + 1],
                                      in_=psum_kptv[:, 2 * cc, 48:49])
                nc.vector.tensor_copy(out=ks2[64:112, cc, 2 * cc + 1:2 * cc + 2],
                                      in_=tmp_o[:, cc, 48:49])

        # ---------------- phase 2: D, attention, projection ----------------
        rdj = rd[:].rearrange("p (r j) -> p j r", j=8)
        qpj = [qpT[:, cc, :].rearrange("p (r j) -> p j r", j=8)
               for cc in range(4)]
        with tc.tile_pool(name="pd", bufs=2, space="PSUM") as pd_p, \
             tc.tile_pool(name="pa", bufs=4, space="PSUM") as pa_p, \
             tc.tile_pool(name="pz", bufs=2, space="PSUM") as pz_p:
            for j in range(8):
                pd = pd_p.tile([8, 512], F32)
                for cc in range(4):
                    nc.tensor.matmul(pd[:], ks2[:, cc, :], qpj[cc][:, j, :],
                                     start=(cc == 0), stop=(cc == 3))
                with nc.allow_low_precision(reason="1/D in bf16 is plenty"):
                    nc.vector.reciprocal(rdj[:, j, :], pd[:])

            # replicate 1/D to the quadrant homes (bf16, one DMA per head);
            # heads 0/1 gate the first attention head so they go first on
            # separate queues.
            for h in range(H):
                q0 = 64 * (h % 2)
                eng = (nc.sync, nc.scalar, nc.gpsimd)[h % 3]
                eng.dma_start(out=rbig[q0:q0 + 64, h // 2, :],
                              in_=_rep_row(rd[h:h + 1, :], 64))

            def norm_cc(cc):
                # pre-normalize qp by 1/D in place (bf16: 2x DVE throughput)
                nc.vector.tensor_mul(qpT[:, cc, :], qpT[:, cc, :],
                                     rbig[:, cc, :])

            def emit_attn_head(h):
                cc, q0 = h // 2, 64 * (h % 2)
                at = at0 if h % 2 == 0 else at1
                kv = kptv_sb[q0:q0 + 48, cc, 0:48]
                qh = qpT[q0:q0 + 48, cc, :].rearrange("p (r j) -> p j r", j=8)
                for jp in range(4):
                    pa = pa_p.tile([128, 512], F32, tag="pa")
                    nc.tensor.matmul(pa[0:48, :], kv, qh[:, 2 * jp, :],
                                     start=True, stop=True,
                                     tile_position=(q0, 0))
                    nc.tensor.matmul(pa[64:112, :], kv, qh[:, 2 * jp + 1, :],
                                     start=True, stop=True,
                                     tile_position=(q0, 64))
                    e0, e1 = ((nc.scalar, nc.vector) if jp % 2 == 0
                              else (nc.vector, nc.scalar))
                    _copy(e0, at[0:48, jp, :], pa[0:48, :])
                    _copy(e1, at[64:112, jp, :], pa[64:112, :])
                return at

            def emit_proj_head(h, at, split_last=False):
                for rc in range(4):
                    pz = pz_p.tile([128, C], F32)
                    for cc in range(4):
                        nc.tensor.matmul(pz[:], at[:, cc, 128 * rc:128 * (rc + 1)],
                                         wp_sb[:, cc, :],
                                         start=(cc == 0), stop=(cc == 3))
                    zo = zo_p.tile([128, C], F32)
                    r0 = 512 * h + 128 * rc
                    zeng = nc.vector if rc % 4 == 1 else nc.scalar
                    _copy(zeng, zo[:], pz[:])
                    nc.sync.dma_start(out=out[r0:r0 + 128, :], in_=zo[:])

            norm_cc(0)
            prev = emit_attn_head(0)
            for h in range(1, H):
                if h % 2 == 1 and h < H - 1:
                    norm_cc((h + 1) // 2)
                cur = emit_attn_head(h)
                emit_proj_head(h - 1, prev)
                prev = cur
            emit_proj_head(H - 1, prev, split_last=True)
    nc.finalize()
    return nc


def _prep_weights(Wqkv, Wproj, bproj):
    """Host-side weight prep: fold dn, pad head dims, build device layouts."""
    Wq = Wqkv[0:C, :]
    Wk = Wqkv[C:2 * C, :]
    Wv = Wqkv[2 * C:3 * C, :]
    wq = np.zeros((C, 512), np.float32)
    for h in range(H):
        wq[:, 64 * h:64 * h + 48] = (DN * Wq[48 * h:48 * (h + 1), :]).T
    wq = np.ascontiguousarray(
        wq.reshape(3, 128, 512)).astype(ml_dtypes.bfloat16)
    wkv = np.concatenate([(DN * Wk).T, Wv.T], axis=1).astype(np.float32)
    wkv = np.ascontiguousarray(
        wkv.reshape(3, 128, 768)).astype(ml_dtypes.bfloat16)
    # wp [128, 4, C]: row p<48 -> slot j=2*cc; row 64<=p<112 -> slot j=2*cc+1
    wp = np.zeros((128, 4, C), np.float32)
    WprojT = np.ascontiguousarray(Wproj.T)
    for cc in range(4):
        wp[0:48, cc, :] = WprojT[48 * (2 * cc):48 * (2 * cc) + 48, :]
        wp[64:112, cc, :] = WprojT[48 * (2 * cc + 1):48 * (2 * cc + 1) + 48, :]
    wp[127, 3, :] = bproj
    return (np.ascontiguousarray(wq.transpose(1, 0, 2)),
            np.ascontiguousarray(wkv.transpose(1, 0, 2)),
            wp.astype(ml_dtypes.bfloat16))


def _prep_x(xb):
    """x [N, C] f32 -> xT [128, 3, N] bf16 (c = kc*128 + p)."""
    xt = np.ascontiguousarray(
        xb.T.reshape(3, 128, N).transpose(1, 0, 2)).astype(ml_dtypes.bfloat16)
    return xt


def _run(inputs, trace=False):
    from concourse.bass_utils import run_bass_kernel_spmd

    x = np.asarray(inputs["x"], dtype=np.float32)
    Wqkv = np.asarray(inputs["Wqkv"], dtype=np.float32)
    Wproj = np.asarray(inputs["Wproj"], dtype=np.float32)
    bproj = np.asarray(inputs["bproj"], dtype=np.float32)
    wq, wkv, wp = _prep_weights(Wqkv, Wproj, bproj)

    if "nc" not in _NC_CACHE:
        _NC_CACHE["nc"] = build_nc()
    nc = _NC_CACHE["nc"]

    in_maps = [
        {"xt": _prep_x(x[b]), "wq": wq, "wkv": wkv, "wp": wp}
        for b in range(B)
    ]
    res = run_bass_kernel_spmd(nc, in_maps, list(range(8)), trace=trace)
    out = np.stack([res.results[b]["out"] for b in range(B)], axis=0)
    return out, res


def kernel(**inputs) -> np.ndarray:
    out, _ = _run(inputs, trace=False)
    return out


def kernel_profiled(**inputs):
    out, res = _run(inputs, trace=True)
    return out, res


# revision 39
# speedup vs baseline: 1.3242x; 1.0157x over previous
"""Trainium2 Bass kernel for nn_Attention_84679575208344 (Performer-style
linear attention). Data-parallel over batch: 8 batches -> 8 NeuronCores.

Math per batch b (reference):
  qkv = x @ Wqkv.T -> split q,k,v per head (HD=48)
  qp = relu(dn*q)+1e-3 ; kp = relu(dn*k)+1e-3          (dn = 48**-0.25)
  ks = kp.sum(n) ; D = qp @ ks ; kptv = v.T @ kp (per head)
  attn = (qp @ kptv.T) / (D + 1e-8)
  out  = reshape(B,H,N,HD)->(B,N,C) WITHOUT head transpose, then @ Wproj.T + b

Layout strategy:
  - x is transposed on the host and uploaded bf16 as xT [128, 3, N]
    (c = kc*128 + p), so no on-device transpose is needed.
  - All moving matmul operands are bf16 (1 cycle/row in all cases).
  - Heads live in 64-aligned partition quadrants: head h sits at partitions
    64*(h%2)..+48 of slab h//2.  Attention j-even/j-odd outputs go to PSUM
    partitions 0:48 / 64:112 of one bank via tile_position, so one copy
    assembles a j-pair.
  - ks rides as an extra column of v through the kptv matmul (output
    partitions are free).  D = ks . qp via small packed matmuls; 1/D is
    replicated across partitions with 8 large SBUF DMAs (bf16) and qp is
    pre-normalized in place (bf16 2x DVE mode).
  - The projection reads head-padded at tiles [128, 4, 512]; pad rows hit
    zero rows of wp, row 127 is constant 1.0 and row 127 of wp slab 3 holds
    the bias.
"""

from contextlib import ExitStack

import numpy as np
import ml_dtypes

import concourse.bass as bass
import concourse.mybir as mybir
import concourse.tile as tile
from concourse import bacc

F32 = mybir.dt.float32
F32R = mybir.dt.float32r
BF16 = mybir.dt.bfloat16
AL = mybir.AluOpType

B, N, C, H = 8, 4096, 384, 8
HD = 48
KEPS = 1e-3
DN = float(HD ** (-0.25))
NCHUNK = N // 128  # 32
NBLK = N // 512    # 8

_NC_CACHE = {}


def _copy(eng, out, in_):
    if hasattr(eng, "copy"):
        eng.copy(out=out, in_=in_)
    else:
        eng.tensor_copy(out=out, in_=in_)


def _rep_row(src_ap, n):
    """Replicate a [1, F] SBUF row AP to n rows via a zero-step middle dim."""
    return bass.AP(tensor=src_ap.tensor, offset=src_ap.offset,
                   ap=[src_ap.ap[0], [0, n], src_ap.ap[1]])


def build_nc():
    nc = bacc.Bacc("TRN2", target_bir_lowering=False, debug=False, num_devices=8)
    xt = nc.declare_dram_parameter("xt", [128, 3, N], BF16, isOutput=False)
    wq = nc.declare_dram_parameter("wq", [128, 3, 512], BF16, isOutput=False)
    wkv = nc.declare_dram_parameter("wkv", [128, 3, 768], BF16, isOutput=False)
    wp = nc.declare_dram_parameter("wp", [128, 4, C], BF16, isOutput=False)
    out = nc.declare_dram_parameter("out", [N, C], F32, isOutput=True)

    with tile.TileContext(nc) as tc, ExitStack() as ctx:
        persist = ctx.enter_context(tc.tile_pool(name="persist", bufs=1))
        kp_p = ctx.enter_context(tc.tile_pool(name="kp", bufs=2))
        v_p = ctx.enter_context(tc.tile_pool(name="v", bufs=3))
        zo_p = ctx.enter_context(tc.tile_pool(name="zo", bufs=3))

        xT = persist.tile([128, 3, N], BF16)
        wq_sb = persist.tile([128, 3, 512], BF16)
        wkv_sb = persist.tile([128, 3, 768], BF16)
        wp_sb = persist.tile([128, 4, C], BF16)
        qpT = persist.tile([128, 4, N], BF16)    # head h at [64*(h%2)+d, h//2]
        kptv_sb = persist.tile([128, 4, 49], BF16)  # col 48 = ks
        ks2 = persist.tile([128, 4, 8], BF16)    # ks: slab cc col h, zero-masked
        rd = persist.tile([8, N], BF16)          # 1/D per head (row h)
        rbig = persist.tile([128, 4, N], BF16)   # 1/D replicated to quadrants
        at0 = persist.tile([128, 4, 512], BF16, tag="at0")
        at1 = persist.tile([128, 4, 512], BF16, tag="at1")

        # weight + input loads; wkv and xt block 0 gate the first matmul, so
        # they go first on separate queues
        nc.sync.dma_start(out=wkv_sb[:], in_=wkv[:])
        nc.scalar.dma_start(out=xT[:, :, 0:512], in_=xt[:, :, 0:512])
        nc.gpsimd.dma_start(out=wq_sb[:], in_=wq[:])
        xt_eng = (nc.sync, nc.scalar, nc.gpsimd)
        for b in range(1, NBLK):
            bs = slice(512 * b, 512 * (b + 1))
            xt_eng[b % 3].dma_start(out=xT[:, :, bs], in_=xt[:, :, bs])
        nc.sync.dma_start(out=wp_sb[:], in_=wp[:])

        nc.vector.memset(ks2[:], 0.0)
        # at pad rows: 48:64 and 112:127 must be finite (they hit zero wp
        # rows); row 127 is the bias row: 1.0 on slab 3, 0 elsewhere.
        # Engine writes need 32-aligned partition bases; row 127 goes via DMA.
        one_row = persist.tile([1, 512], BF16)
        nc.vector.memset(one_row[:], 1.0)
        for at in (at0, at1):
            nc.gpsimd.memset(at[32:64, :, :], 0.0)
            nc.gpsimd.memset(at[96:128, :, :], 0.0)
        nc.sync.dma_start(out=at0[127:128, 3, :], in_=one_row[:])
        nc.scalar.dma_start(out=at1[127:128, 3, :], in_=one_row[:])

        # ---------------- phase 1: K/V, kptv(+ks), qp^T ----------------
        with tc.tile_pool(name="pq", bufs=3, space="PSUM") as pq_p, \
             tc.tile_pool(name="pkv", bufs=2, space="PSUM") as pkv_p, \
             tc.tile_pool(name="pkp", bufs=1, space="PSUM") as pkp_p:
            # CoreSim's psum-group model only tracks partition-base-0 APs (or
            # exact [128,512] single-bank tiles) coherently, so kptv
            # accumulates at base 0 with heads along the free dim and is
            # remapped to quadrant homes afterwards.
            psum_kptv = pkp_p.tile([48, 8, 49], F32)

            def emit_q_block(blk):
                bs = slice(512 * blk, 512 * (blk + 1))
                for mc in range(4):
                    pq = pq_p.tile([128, 512], F32, tag="pq")
                    for kc in range(3):
                        nc.tensor.matmul(pq[:], wq_sb[:, kc, 128 * mc:128 * (mc + 1)],
                                         xT[:, kc, bs],
                                         start=(kc == 0), stop=(kc == 2))
                    nc.vector.tensor_scalar(qpT[:, mc, bs], pq[:], 0.0, KEPS,
                                      op0=AL.max, op1=AL.add)

            for i in range(NCHUNK):
                ns = slice(128 * i, 128 * (i + 1))
                pkv = pkv_p.tile([128, 768], F32)
                for kc in range(3):
                    lhs = xT[:, kc, ns]
                    nc.tensor.matmul(pkv[:, 0:512], lhs, wkv_sb[:, kc, 0:512],
                                     start=(kc == 0), stop=(kc == 2))
                    nc.tensor.matmul(pkv[:, 512:768], lhs, wkv_sb[:, kc, 512:768],
                                     start=(kc == 0), stop=(kc == 2))
                kp = kp_p.tile([128, C], BF16)
                nc.vector.tensor_scalar(kp[:], pkv[:, 0:C], 0.0, KEPS,
                                   op0=AL.max, op1=AL.add)
                v = v_p.tile([128, 8, 49], BF16, tag="v")
                veng = (nc.scalar, nc.scalar, nc.vector)[i % 3]
                _copy(veng, v[:, :, 0:48],
                      pkv[:, C:768].rearrange("p (h d) -> p h d", h=8))
                nc.gpsimd.memset(v[:, :, 48:49], 1.0)
                for h in range(H):
                    nc.tensor.matmul(psum_kptv[:, h, :],
                                     kp[:, 48 * h:48 * (h + 1)], v[:, h, :],
                                     start=(i == 0 and h == 0),
                                     stop=(i == NCHUNK - 1 and h == H - 1))
                if i % 4 == 3:
                    emit_q_block(i // 4)

            # kptv psum -> quadrant homes: even heads copy in place, odd heads
            # hop partitions via a small SBUF-to-SBUF DMA.  ks rides col 48.
            tmp_o = kp_p.tile([48, 4, 49], BF16)
            nc.scalar.copy(out=kptv_sb[0:48, :, :], in_=psum_kptv[:, 0::2, :])
            nc.vector.tensor_copy(out=tmp_o[:], in_=psum_kptv[:, 1::2, :])
            nc.sync.dma_start(out=kptv_sb[64:112, :, :], in_=tmp_o[:])
            for cc in range(4):
                nc.vector.tensor_copy(out=ks2[0:48, cc, 2 * cc:2 * cc + 1],
                                      in_=psum_kptv[:, 2 * cc, 48:49])
                nc.vector.tensor_copy(out=ks2[64:112, cc, 2 * cc + 1:2 * cc + 2],
                                      in_=tmp_o[:, cc, 48:49])

        # ---------------- phase 2: D, attention, projection ----------------
        rdj = rd[:].rearrange("p (r j) -> p j r", j=8)
        qpj = [qpT[:, cc, :].rearrange("p (r j) -> p j r", j=8)
               for cc in range(4)]
        with tc.tile_pool(name="pd", bufs=2, space="PSUM") as pd_p:
            for j in range(8):
                pd = pd_p.tile([8, 512], F32)
                for cc in range(4):
                    nc.tensor.matmul(pd[:], ks2[:, cc, :], qpj[cc][:, j, :],
                                     start=(cc == 0), stop=(cc == 3))
                with nc.allow_low_precision(reason="1/D in bf16 is plenty"):
                    nc.vector.reciprocal(rdj[:, j, :], pd[:])

        # replicate 1/D to the quadrant homes (bf16, one DMA per head);
        # heads 0/1 gate the first attention head so they go first on
        # separate queues.
        for h in range(H):
            q0 = 64 * (h % 2)
            eng = (nc.sync, nc.scalar, nc.gpsimd)[h % 3]
            eng.dma_start(out=rbig[q0:q0 + 64, h // 2, :],
                          in_=_rep_row(rd[h:h + 1, :], 64))

        with tc.tile_pool(name="pa", bufs=6, space="PSUM") as pa_p, \
             tc.tile_pool(name="pz", bufs=2, space="PSUM") as pz_p:

            def norm_cc(cc):
                # pre-normalize qp by 1/D in place (bf16: 2x DVE throughput)
                nc.vector.tensor_mul(qpT[:, cc, :], qpT[:, cc, :],
                                     rbig[:, cc, :])

            def emit_attn_head(h):
                cc, q0 = h // 2, 64 * (h % 2)
                at = at0 if h % 2 == 0 else at1
                kv = kptv_sb[q0:q0 + 48, cc, 0:48]
                qh = qpT[q0:q0 + 48, cc, :].rearrange("p (r j) -> p j r", j=8)
                for jp in range(4):
                    pa = pa_p.tile([128, 512], F32, tag="pa")
                    nc.tensor.matmul(pa[0:48, :], kv, qh[:, 2 * jp, :],
                                     start=True, stop=True,
                                     tile_position=(q0, 0))
                    nc.tensor.matmul(pa[64:112, :], kv, qh[:, 2 * jp + 1, :],
                                     start=True, stop=True,
                                     tile_position=(q0, 64))
                    e0, e1 = ((nc.scalar, nc.vector) if jp % 2 == 0
                              else (nc.vector, nc.scalar))
                    _copy(e0, at[0:48, jp, :], pa[0:48, :])
                    _copy(e1, at[64:112, jp, :], pa[64:112, :])
                return at

            def emit_proj_head(h, at, split_last=False):
                for rc in range(4):
                    pz = pz_p.tile([128, C], F32)
                    for cc in range(4):
                        nc.tensor.matmul(pz[:], at[:, cc, 128 * rc:128 * (rc + 1)],
                                         wp_sb[:, cc, :],
                                         start=(cc == 0), stop=(cc == 3))
                    zo = zo_p.tile([128, C], F32)
                    r0 = 512 * h + 128 * rc
                    zeng = nc.vector if rc % 4 == 1 else nc.scalar
                    _copy(zeng, zo[:], pz[:])
                    nc.sync.dma_start(out=out[r0:r0 + 128, :], in_=zo[:])

            norm_cc(0)
            prev = emit_attn_head(0)
            for h in range(1, H):
                if h % 2 == 1 and h < H - 1:
                    norm_cc((h + 1) // 2)
                cur = emit_attn_head(h)
                emit_proj_head(h - 1, prev)
                prev = cur
            emit_proj_head(H - 1, prev, split_last=True)
    nc.finalize()
    return nc


def _prep_weights(Wqkv, Wproj, bproj):
    """Host-side weight prep: fold dn, pad head dims, build device layouts."""
    Wq = Wqkv[0:C, :]
    Wk = Wqkv[C:2 * C, :]
    Wv = Wqkv[2 * C:3 * C, :]
    wq = np.zeros((C, 512), np.float32)
    for h in range(H):
        wq[:, 64 * h:64 * h + 48] = (DN * Wq[48 * h:48 * (h + 1), :]).T
    wq = np.ascontiguousarray(
        wq.reshape(3, 128, 512)).astype(ml_dtypes.bfloat16)
    wkv = np.concatenate([(DN * Wk).T, Wv.T], axis=1).astype(np.float32)
    wkv = np.ascontiguousarray(
        wkv.reshape(3, 128, 768)).astype(ml_dtypes.bfloat16)
    # wp [128, 4, C]: row p<48 -> slot j=2*cc; row 64<=p<112 -> slot j=2*cc+1
    wp = np.zeros((128, 4, C), np.float32)
    WprojT = np.ascontiguousarray(Wproj.T)
    for cc in range(4):
        wp[0:48, cc, :] = WprojT[48 * (2 * cc):48 * (2 * cc) + 48, :]
        wp[64:112, cc, :] = WprojT[48 * (2 * cc + 1):48 * (2 * cc + 1) + 48, :]
    wp[127, 3, :] = bproj
    return (np.ascontiguousarray(wq.transpose(1, 0, 2)),
            np.ascontiguousarray(wkv.transpose(1, 0, 2)),
            wp.astype(ml_dtypes.bfloat16))


def _prep_x(xb):
    """x [N, C] f32 -> xT [128, 3, N] bf16 (c = kc*128 + p)."""
    xt = np.ascontiguousarray(
        xb.T.reshape(3, 128, N).transpose(1, 0, 2)).astype(ml_dtypes.bfloat16)
    return xt


def _run(inputs, trace=False):
    from concourse.bass_utils import run_bass_kernel_spmd

    x = np.asarray(inputs["x"], dtype=np.float32)
    Wqkv = np.asarray(inputs["Wqkv"], dtype=np.float32)
    Wproj = np.asarray(inputs["Wproj"], dtype=np.float32)
    bproj = np.asarray(inputs["bproj"], dtype=np.float32)
    wq, wkv, wp = _prep_weights(Wqkv, Wproj, bproj)

    if "nc" not in _NC_CACHE:
        _NC_CACHE["nc"] = build_nc()
    nc = _NC_CACHE["nc"]

    in_maps = [
        {"xt": _prep_x(x[b]), "wq": wq, "wkv": wkv, "wp": wp}
        for b in range(B)
    ]
    res = run_bass_kernel_spmd(nc, in_maps, list(range(8)), trace=trace)
    out = np.stack([res.results[b]["out"] for b in range(B)], axis=0)
    return out, res


def kernel(**inputs) -> np.ndarray:
    out, _ = _run(inputs, trace=False)
    return out


def kernel_profiled(**inputs):
    out, res = _run(inputs, trace=True)
    return out, res


# revision 40
# speedup vs baseline: 1.3346x; 1.0078x over previous
"""Trainium2 Bass kernel for nn_Attention_84679575208344 (Performer-style
linear attention). Data-parallel over batch: 8 batches -> 8 NeuronCores.

Math per batch b (reference):
  qkv = x @ Wqkv.T -> split q,k,v per head (HD=48)
  qp = relu(dn*q)+1e-3 ; kp = relu(dn*k)+1e-3          (dn = 48**-0.25)
  ks = kp.sum(n) ; D = qp @ ks ; kptv = v.T @ kp (per head)
  attn = (qp @ kptv.T) / (D + 1e-8)
  out  = reshape(B,H,N,HD)->(B,N,C) WITHOUT head transpose, then @ Wproj.T + b

Layout strategy:
  - x is transposed on the host and uploaded bf16 as xT [128, 3, N]
    (c = kc*128 + p), so no on-device transpose is needed.
  - All moving matmul operands are bf16 (1 cycle/row in all cases).
  - Heads live in 64-aligned partition quadrants: head h sits at partitions
    64*(h%2)..+48 of slab h//2.  Attention j-even/j-odd outputs go to PSUM
    partitions 0:48 / 64:112 of one bank via tile_position, so one copy
    assembles a j-pair.
  - ks rides as an extra column of v through the kptv matmul (output
    partitions are free).  D = ks . qp via small packed matmuls; 1/D is
    replicated across partitions with 8 large SBUF DMAs (bf16) and qp is
    pre-normalized in place (bf16 2x DVE mode).
  - The projection reads head-padded at tiles [128, 4, 512]; pad rows hit
    zero rows of wp, row 127 is constant 1.0 and row 127 of wp slab 3 holds
    the bias.
"""

from contextlib import ExitStack

import numpy as np
import ml_dtypes

import concourse.bass as bass
import concourse.mybir as mybir
import concourse.tile as tile
from concourse import bacc

F32 = mybir.dt.float32
F32R = mybir.dt.float32r
BF16 = mybir.dt.bfloat16
AL = mybir.AluOpType

B, N, C, H = 8, 4096, 384, 8
HD = 48
KEPS = 1e-3
DN = float(HD ** (-0.25))
NCHUNK = N // 128  # 32
NBLK = N // 512    # 8

_NC_CACHE = {}


def _copy(eng, out, in_):
    if hasattr(eng, "copy"):
        eng.copy(out=out, in_=in_)
    else:
        eng.tensor_copy(out=out, in_=in_)


def _rep_row(src_ap, n):
    """Replicate a [1, F] SBUF row AP to n rows via a zero-step middle dim."""
    return bass.AP(tensor=src_ap.tensor, offset=src_ap.offset,
                   ap=[src_ap.ap[0], [0, n], src_ap.ap[1]])


def build_nc():
    nc = bacc.Bacc("TRN2", target_bir_lowering=False, debug=False, num_devices=8)
    xt = nc.declare_dram_parameter("xt", [128, 3, N], BF16, isOutput=False)
    wq = nc.declare_dram_parameter("wq", [128, 3, 512], BF16, isOutput=False)
    wkv = nc.declare_dram_parameter("wkv", [128, 3, 768], BF16, isOutput=False)
    wp = nc.declare_dram_parameter("wp", [128, 4, C], BF16, isOutput=False)
    out = nc.declare_dram_parameter("out", [N, C], F32, isOutput=True)

    with tile.TileContext(nc) as tc, ExitStack() as ctx:
        persist = ctx.enter_context(tc.tile_pool(name="persist", bufs=1))
        kp_p = ctx.enter_context(tc.tile_pool(name="kp", bufs=3))
        v_p = ctx.enter_context(tc.tile_pool(name="v", bufs=4))
        zo_p = ctx.enter_context(tc.tile_pool(name="zo", bufs=4))

        xT = persist.tile([128, 3, N], BF16)
        wq_sb = persist.tile([128, 3, 512], BF16)
        wkv_sb = persist.tile([128, 3, 768], BF16)
        wp_sb = persist.tile([128, 4, C], BF16)
        qpT = persist.tile([128, 4, N], BF16)    # head h at [64*(h%2)+d, h//2]
        kptv_sb = persist.tile([128, 4, 49], BF16)  # col 48 = ks
        ks2 = persist.tile([128, 4, 8], BF16)    # ks: slab cc col h, zero-masked
        rd = persist.tile([8, N], BF16)          # 1/D per head (row h)
        rbig = persist.tile([128, 4, N], BF16)   # 1/D replicated to quadrants
        at0 = persist.tile([128, 4, 512], BF16, tag="at0")
        at1 = persist.tile([128, 4, 512], BF16, tag="at1")

        # weight + input loads; wkv and xt block 0 gate the first matmul, so
        # they go first on separate queues
        nc.sync.dma_start(out=wkv_sb[:], in_=wkv[:])
        nc.scalar.dma_start(out=xT[:, :, 0:512], in_=xt[:, :, 0:512])
        nc.gpsimd.dma_start(out=wq_sb[:], in_=wq[:])
        xt_eng = (nc.sync, nc.scalar, nc.gpsimd)
        for b in range(1, NBLK):
            bs = slice(512 * b, 512 * (b + 1))
            xt_eng[b % 3].dma_start(out=xT[:, :, bs], in_=xt[:, :, bs])
        nc.sync.dma_start(out=wp_sb[:], in_=wp[:])

        nc.vector.memset(ks2[:], 0.0)
        # at pad rows: 48:64 and 112:127 must be finite (they hit zero wp
        # rows); row 127 is the bias row: 1.0 on slab 3, 0 elsewhere.
        # Engine writes need 32-aligned partition bases; row 127 goes via DMA.
        one_row = persist.tile([1, 512], BF16)
        nc.vector.memset(one_row[:], 1.0)
        for at in (at0, at1):
            nc.gpsimd.memset(at[32:64, :, :], 0.0)
            nc.gpsimd.memset(at[96:128, :, :], 0.0)
        nc.sync.dma_start(out=at0[127:128, 3, :], in_=one_row[:])
        nc.scalar.dma_start(out=at1[127:128, 3, :], in_=one_row[:])

        # ---------------- phase 1: K/V, kptv(+ks), qp^T ----------------
        with tc.tile_pool(name="pq", bufs=3, space="PSUM") as pq_p, \
             tc.tile_pool(name="pkv", bufs=2, space="PSUM") as pkv_p, \
             tc.tile_pool(name="pkp", bufs=1, space="PSUM") as pkp_p:
            # CoreSim's psum-group model only tracks partition-base-0 APs (or
            # exact [128,512] single-bank tiles) coherently, so kptv
            # accumulates at base 0 with heads along the free dim and is
            # remapped to quadrant homes afterwards.
            psum_kptv = pkp_p.tile([48, 8, 49], F32)

            def emit_q_block(blk):
                bs = slice(512 * blk, 512 * (blk + 1))
                for mc in range(4):
                    pq = pq_p.tile([128, 512], F32, tag="pq")
                    for kc in range(3):
                        nc.tensor.matmul(pq[:], wq_sb[:, kc, 128 * mc:128 * (mc + 1)],
                                         xT[:, kc, bs],
                                         start=(kc == 0), stop=(kc == 2))
                    nc.vector.tensor_scalar(qpT[:, mc, bs], pq[:], 0.0, KEPS,
                                      op0=AL.max, op1=AL.add)

            for i in range(NCHUNK):
                ns = slice(128 * i, 128 * (i + 1))
                pkv = pkv_p.tile([128, 768], F32)
                for kc in range(3):
                    lhs = xT[:, kc, ns]
                    nc.tensor.matmul(pkv[:, 0:512], lhs, wkv_sb[:, kc, 0:512],
                                     start=(kc == 0), stop=(kc == 2))
                    nc.tensor.matmul(pkv[:, 512:768], lhs, wkv_sb[:, kc, 512:768],
                                     start=(kc == 0), stop=(kc == 2))
                kp = kp_p.tile([128, C], BF16)
                nc.vector.tensor_scalar(kp[:], pkv[:, 0:C], 0.0, KEPS,
                                   op0=AL.max, op1=AL.add)
                v = v_p.tile([128, 8, 49], BF16, tag="v")
                veng = (nc.scalar, nc.scalar, nc.vector)[i % 3]
                _copy(veng, v[:, :, 0:48],
                      pkv[:, C:768].rearrange("p (h d) -> p h d", h=8))
                nc.gpsimd.memset(v[:, :, 48:49], 1.0)
                for h in range(H):
                    nc.tensor.matmul(psum_kptv[:, h, :],
                                     kp[:, 48 * h:48 * (h + 1)], v[:, h, :],
                                     start=(i == 0 and h == 0),
                                     stop=(i == NCHUNK - 1 and h == H - 1))
                if i % 4 == 3:
                    emit_q_block(i // 4)

            # kptv psum -> quadrant homes: even heads copy in place, odd heads
            # hop partitions via a small SBUF-to-SBUF DMA.  ks rides col 48.
            tmp_o = kp_p.tile([48, 4, 49], BF16)
            nc.scalar.copy(out=kptv_sb[0:48, :, :], in_=psum_kptv[:, 0::2, :])
            nc.vector.tensor_copy(out=tmp_o[:], in_=psum_kptv[:, 1::2, :])
            nc.sync.dma_start(out=kptv_sb[64:112, :, :], in_=tmp_o[:])
            for cc in range(4):
                nc.vector.tensor_copy(out=ks2[0:48, cc, 2 * cc:2 * cc + 1],
                                      in_=psum_kptv[:, 2 * cc, 48:49])
                nc.vector.tensor_copy(out=ks2[64:112, cc, 2 * cc + 1:2 * cc + 2],
                                      in_=tmp_o[:, cc, 48:49])

        # ---------------- phase 2: D, attention, projection ----------------
        rdj = rd[:].rearrange("p (r j) -> p j r", j=8)
        qpj = [qpT[:, cc, :].rearrange("p (r j) -> p j r", j=8)
               for cc in range(4)]
        with tc.tile_pool(name="pd", bufs=2, space="PSUM") as pd_p:
            for j in range(8):
                pd = pd_p.tile([8, 512], F32)
                for cc in range(4):
                    nc.tensor.matmul(pd[:], ks2[:, cc, :], qpj[cc][:, j, :],
                                     start=(cc == 0), stop=(cc == 3))
                with nc.allow_low_precision(reason="1/D in bf16 is plenty"):
                    nc.vector.reciprocal(rdj[:, j, :], pd[:])

        # replicate 1/D to the quadrant homes (bf16, one DMA per head);
        # heads 0/1 gate the first attention head so they go first on
        # separate queues.
        for h in range(H):
            q0 = 64 * (h % 2)
            eng = (nc.sync, nc.scalar, nc.gpsimd)[h % 3]
            eng.dma_start(out=rbig[q0:q0 + 64, h // 2, :],
                          in_=_rep_row(rd[h:h + 1, :], 64))

        with tc.tile_pool(name="pa", bufs=6, space="PSUM") as pa_p, \
             tc.tile_pool(name="pz", bufs=2, space="PSUM") as pz_p:

            def norm_cc(cc):
                # pre-normalize qp by 1/D in place (bf16: 2x DVE throughput)
                nc.vector.tensor_mul(qpT[:, cc, :], qpT[:, cc, :],
                                     rbig[:, cc, :])

            def emit_attn_head(h):
                cc, q0 = h // 2, 64 * (h % 2)
                at = at0 if h % 2 == 0 else at1
                kv = kptv_sb[q0:q0 + 48, cc, 0:48]
                qh = qpT[q0:q0 + 48, cc, :].rearrange("p (r j) -> p j r", j=8)
                for jp in range(4):
                    pa = pa_p.tile([128, 512], F32, tag="pa")
                    nc.tensor.matmul(pa[0:48, :], kv, qh[:, 2 * jp, :],
                                     start=True, stop=True,
                                     tile_position=(q0, 0))
                    nc.tensor.matmul(pa[64:112, :], kv, qh[:, 2 * jp + 1, :],
                                     start=True, stop=True,
                                     tile_position=(q0, 64))
                    e0, e1 = ((nc.scalar, nc.vector) if jp % 2 == 0
                              else (nc.vector, nc.scalar))
                    _copy(e0, at[0:48, jp, :], pa[0:48, :])
                    _copy(e1, at[64:112, jp, :], pa[64:112, :])
                return at

            def emit_proj_head(h, at, split_last=False):
                for rc in range(4):
                    pz = pz_p.tile([128, C], F32)
                    for cc in range(4):
                        nc.tensor.matmul(pz[:], at[:, cc, 128 * rc:128 * (rc + 1)],
                                         wp_sb[:, cc, :],
                                         start=(cc == 0), stop=(cc == 3))
                    zo = zo_p.tile([128, C], F32)
                    r0 = 512 * h + 128 * rc
                    zeng = nc.vector if rc % 4 == 1 else nc.scalar
                    _copy(zeng, zo[:], pz[:])
                    nc.sync.dma_start(out=out[r0:r0 + 128, :], in_=zo[:])

            norm_cc(0)
            prev = emit_attn_head(0)
            for h in range(1, H):
                if h % 2 == 1 and h < H - 1:
                    norm_cc((h + 1) // 2)
                cur = emit_attn_head(h)
                emit_proj_head(h - 1, prev)
                prev = cur
            emit_proj_head(H - 1, prev, split_last=True)
    nc.finalize()
    return nc


def _prep_weights(Wqkv, Wproj, bproj):
    """Host-side weight prep: fold dn, pad head dims, build device layouts."""
    Wq = Wqkv[0:C, :]
    Wk = Wqkv[C:2 * C, :]
    Wv = Wqkv[2 * C:3 * C, :]
    wq = np.zeros((C, 512), np.float32)
    for h in range(H):
        wq[:, 64 * h:64 * h + 48] = (DN * Wq[48 * h:48 * (h + 1), :]).T
    wq = np.ascontiguousarray(
        wq.reshape(3, 128, 512)).astype(ml_dtypes.bfloat16)
    wkv = np.concatenate([(DN * Wk).T, Wv.T], axis=1).astype(np.float32)
    wkv = np.ascontiguousarray(
        wkv.reshape(3, 128, 768)).astype(ml_dtypes.bfloat16)
    # wp [128, 4, C]: row p<48 -> slot j=2*cc; row 64<=p<112 -> slot j=2*cc+1
    wp = np.zeros((128, 4, C), np.float32)
    WprojT = np.ascontiguousarray(Wproj.T)
    for cc in range(4):
        wp[0:48, cc, :] = WprojT[48 * (2 * cc):48 * (2 * cc) + 48, :]
        wp[64:112, cc, :] = WprojT[48 * (2 * cc + 1):48 * (2 * cc + 1) + 48, :]
    wp[127, 3, :] = bproj
    return (np.ascontiguousarray(wq.transpose(1, 0, 2)),
            np.ascontiguousarray(wkv.transpose(1, 0, 2)),
            wp.astype(ml_dtypes.bfloat16))


def _prep_x(xb):
    """x [N, C] f32 -> xT [128, 3, N] bf16 (c = kc*128 + p)."""
    xt = np.ascontiguousarray(
        xb.T.reshape(3, 128, N).transpose(1, 0, 2)).astype(ml_dtypes.bfloat16)
    return xt


def _run(inputs, trace=False):
    from concourse.bass_utils import run_bass_kernel_spmd

    x = np.asarray(inputs["x"], dtype=np.float32)
    Wqkv = np.asarray(inputs["Wqkv"], dtype=np.float32)
    Wproj = np.asarray(inputs["Wproj"], dtype=np.float32)
    bproj = np.asarray(inputs["bproj"], dtype=np.float32)
    wq, wkv, wp = _prep_weights(Wqkv, Wproj, bproj)

    if "nc" not in _NC_CACHE:
        _NC_CACHE["nc"] = build_nc()
    nc = _NC_CACHE["nc"]

    in_maps = [
        {"xt": _prep_x(x[b]), "wq": wq, "wkv": wkv, "wp": wp}
        for b in range(B)
    ]
    res = run_bass_kernel_spmd(nc, in_maps, list(range(8)), trace=trace)
    out = np.stack([res.results[b]["out"] for b in range(B)], axis=0)
    return out, res


def kernel(**inputs) -> np.ndarray:
    out, _ = _run(inputs, trace=False)
    return out


def kernel_profiled(**inputs):
    out, res = _run(inputs, trace=True)
    return out, res


# revision 44
# speedup vs baseline: 1.3353x; 1.0005x over previous
"""Trainium2 Bass kernel for nn_Attention_84679575208344 (Performer-style
linear attention). Data-parallel over batch: 8 batches -> 8 NeuronCores.

Math per batch b (reference):
  qkv = x @ Wqkv.T -> split q,k,v per head (HD=48)
  qp = relu(dn*q)+1e-3 ; kp = relu(dn*k)+1e-3          (dn = 48**-0.25)
  ks = kp.sum(n) ; D = qp @ ks ; kptv = v.T @ kp (per head)
  attn = (qp @ kptv.T) / (D + 1e-8)
  out  = reshape(B,H,N,HD)->(B,N,C) WITHOUT head transpose, then @ Wproj.T + b

Layout strategy:
  - x is transposed on the host and uploaded bf16 as xT [128, 3, N]
    (c = kc*128 + p), so no on-device transpose is needed.
  - All moving matmul operands are bf16 (1 cycle/row in all cases).
  - Heads live in 64-aligned partition quadrants: head h sits at partitions
    64*(h%2)..+48 of slab h//2.  Attention j-even/j-odd outputs go to PSUM
    partitions 0:48 / 64:112 of one bank via tile_position, so one copy
    assembles a j-pair.
  - ks rides as an extra column of v through the kptv matmul (output
    partitions are free).  D = ks . qp via small packed matmuls; 1/D is
    replicated across partitions with 8 large SBUF DMAs (bf16) and qp is
    pre-normalized in place (bf16 2x DVE mode).
  - The projection reads head-padded at tiles [128, 4, 512]; pad rows hit
    zero rows of wp, row 127 is constant 1.0 and row 127 of wp slab 3 holds
    the bias.
"""

from contextlib import ExitStack

import numpy as np
import ml_dtypes

import concourse.bass as bass
import concourse.mybir as mybir
import concourse.tile as tile
from concourse import bacc

F32 = mybir.dt.float32
F32R = mybir.dt.float32r
BF16 = mybir.dt.bfloat16
AL = mybir.AluOpType

B, N, C, H = 8, 4096, 384, 8
HD = 48
KEPS = 1e-3
DN = float(HD ** (-0.25))
NCHUNK = N // 128  # 32
NBLK = N // 512    # 8

_NC_CACHE = {}


def _copy(eng, out, in_):
    if hasattr(eng, "copy"):
        eng.copy(out=out, in_=in_)
    else:
        eng.tensor_copy(out=out, in_=in_)


def _rep_row(src_ap, n):
    """Replicate a [1, F] SBUF row AP to n rows via a zero-step middle dim."""
    return bass.AP(tensor=src_ap.tensor, offset=src_ap.offset,
                   ap=[src_ap.ap[0], [0, n], src_ap.ap[1]])


def build_nc():
    nc = bacc.Bacc("TRN2", target_bir_lowering=False, debug=False, num_devices=8)
    xt = nc.declare_dram_parameter("xt", [128, 3, N], BF16, isOutput=False)
    wq = nc.declare_dram_parameter("wq", [128, 3, 512], BF16, isOutput=False)
    wkv = nc.declare_dram_parameter("wkv", [128, 3, 768], BF16, isOutput=False)
    wp = nc.declare_dram_parameter("wp", [128, 4, C], BF16, isOutput=False)
    out = nc.declare_dram_parameter("out", [N, C], F32, isOutput=True)

    with tile.TileContext(nc) as tc, ExitStack() as ctx:
        persist = ctx.enter_context(tc.tile_pool(name="persist", bufs=1))
        kp_p = ctx.enter_context(tc.tile_pool(name="kp", bufs=3))
        v_p = ctx.enter_context(tc.tile_pool(name="v", bufs=4))
        zo_p = ctx.enter_context(tc.tile_pool(name="zo", bufs=4))

        xT = persist.tile([128, 3, N], BF16)
        wq_sb = persist.tile([128, 3, 512], BF16)
        wkv_sb = persist.tile([128, 3, 768], BF16)
        wp_sb = persist.tile([128, 4, C], BF16)
        qpT = persist.tile([128, 4, N], BF16)    # head h at [64*(h%2)+d, h//2]
        kptv_sb = persist.tile([128, 4, 49], BF16)  # col 48 = ks
        ks2 = persist.tile([128, 4, 8], BF16)    # ks: slab cc col h, zero-masked
        rd = persist.tile([8, N], BF16)          # 1/D per head (row h)
        rbig = persist.tile([128, 4, N], BF16)   # 1/D replicated to quadrants
        at0 = persist.tile([128, 4, 512], BF16, tag="at0")
        at1 = persist.tile([128, 4, 512], BF16, tag="at1")

        # weight + input loads; wkv and xt block 0 gate the first matmul, so
        # they go first on separate queues
        nc.sync.dma_start(out=wq_sb[:], in_=wq[:])
        nc.scalar.dma_start(out=xT[:, :, 0:512], in_=xt[:, :, 0:512])
        nc.gpsimd.dma_start(out=wkv_sb[:], in_=wkv[:])
        xt_eng = (nc.sync, nc.scalar, nc.gpsimd)
        for b in range(1, NBLK):
            bs = slice(512 * b, 512 * (b + 1))
            xt_eng[b % 3].dma_start(out=xT[:, :, bs], in_=xt[:, :, bs])
        nc.sync.dma_start(out=wp_sb[:], in_=wp[:])

        nc.vector.memset(ks2[:], 0.0)
        # at pad rows: 48:64 and 112:127 must be finite (they hit zero wp
        # rows); row 127 is the bias row: 1.0 on slab 3, 0 elsewhere.
        # Engine writes need 32-aligned partition bases; row 127 goes via DMA.
        one_row = persist.tile([1, 512], BF16)
        nc.vector.memset(one_row[:], 1.0)
        for at in (at0, at1):
            nc.gpsimd.memset(at[32:64, :, :], 0.0)
            nc.gpsimd.memset(at[96:128, :, :], 0.0)
        nc.sync.dma_start(out=at0[127:128, 3, :], in_=one_row[:])
        nc.scalar.dma_start(out=at1[127:128, 3, :], in_=one_row[:])

        # ---------------- phase 1: K/V, kptv(+ks), qp^T ----------------
        with tc.tile_pool(name="pq", bufs=3, space="PSUM") as pq_p, \
             tc.tile_pool(name="pkv", bufs=2, space="PSUM") as pkv_p, \
             tc.tile_pool(name="pkp", bufs=1, space="PSUM") as pkp_p:
            # CoreSim's psum-group model only tracks partition-base-0 APs (or
            # exact [128,512] single-bank tiles) coherently, so kptv
            # accumulates at base 0 with heads along the free dim and is
            # remapped to quadrant homes afterwards.
            psum_kptv = pkp_p.tile([48, 8, 49], F32)

            def emit_q_block(blk):
                bs = slice(512 * blk, 512 * (blk + 1))
                for mc in range(4):
                    pq = pq_p.tile([128, 512], F32, tag="pq")
                    for kc in range(3):
                        nc.tensor.matmul(pq[:], wq_sb[:, kc, 128 * mc:128 * (mc + 1)],
                                         xT[:, kc, bs],
                                         start=(kc == 0), stop=(kc == 2))
                    nc.vector.tensor_scalar(qpT[:, mc, bs], pq[:], 0.0, KEPS,
                                      op0=AL.max, op1=AL.add)

            emit_q_block(0)
            for i in range(NCHUNK):
                ns = slice(128 * i, 128 * (i + 1))
                pkv = pkv_p.tile([128, 768], F32)
                for kc in range(3):
                    lhs = xT[:, kc, ns]
                    nc.tensor.matmul(pkv[:, 0:512], lhs, wkv_sb[:, kc, 0:512],
                                     start=(kc == 0), stop=(kc == 2))
                    nc.tensor.matmul(pkv[:, 512:768], lhs, wkv_sb[:, kc, 512:768],
                                     start=(kc == 0), stop=(kc == 2))
                kp = kp_p.tile([128, C], BF16)
                nc.vector.tensor_scalar(kp[:], pkv[:, 0:C], 0.0, KEPS,
                                   op0=AL.max, op1=AL.add)
                v = v_p.tile([128, 8, 49], BF16, tag="v")
                veng = (nc.scalar, nc.scalar, nc.vector)[i % 3]
                _copy(veng, v[:, :, 0:48],
                      pkv[:, C:768].rearrange("p (h d) -> p h d", h=8))
                nc.gpsimd.memset(v[:, :, 48:49], 1.0)
                for h in range(H):
                    nc.tensor.matmul(psum_kptv[:, h, :],
                                     kp[:, 48 * h:48 * (h + 1)], v[:, h, :],
                                     start=(i == 0 and h == 0),
                                     stop=(i == NCHUNK - 1 and h == H - 1))
                if i % 4 == 3 and i > 3:
                    emit_q_block(i // 4)

            # kptv psum -> quadrant homes: even heads copy in place, odd heads
            # hop partitions via a small SBUF-to-SBUF DMA.  ks rides col 48.
            tmp_o = kp_p.tile([48, 4, 49], BF16)
            nc.scalar.copy(out=kptv_sb[0:48, :, :], in_=psum_kptv[:, 0::2, :])
            nc.vector.tensor_copy(out=tmp_o[:], in_=psum_kptv[:, 1::2, :])
            nc.sync.dma_start(out=kptv_sb[64:112, :, :], in_=tmp_o[:])
            for cc in range(4):
                nc.vector.tensor_copy(out=ks2[0:48, cc, 2 * cc:2 * cc + 1],
                                      in_=psum_kptv[:, 2 * cc, 48:49])
                nc.vector.tensor_copy(out=ks2[64:112, cc, 2 * cc + 1:2 * cc + 2],
                                      in_=tmp_o[:, cc, 48:49])

        # ---------------- phase 2: D, attention, projection ----------------
        rdj = rd[:].rearrange("p (r j) -> p j r", j=8)
        qpj = [qpT[:, cc, :].rearrange("p (r j) -> p j r", j=8)
               for cc in range(4)]
        with tc.tile_pool(name="pd", bufs=2, space="PSUM") as pd_p:
            for j in range(8):
                pd = pd_p.tile([8, 512], F32)
                for cc in range(4):
                    nc.tensor.matmul(pd[:], ks2[:, cc, :], qpj[cc][:, j, :],
                                     start=(cc == 0), stop=(cc == 3))
                with nc.allow_low_precision(reason="1/D in bf16 is plenty"):
                    nc.vector.reciprocal(rdj[:, j, :], pd[:])

        # replicate 1/D to the quadrant homes (bf16, one DMA per head);
        # heads 0/1 gate the first attention head so they go first on
        # separate queues.
        for h in range(H):
            q0 = 64 * (h % 2)
            eng = (nc.sync, nc.scalar, nc.gpsimd)[h % 3]
            eng.dma_start(out=rbig[q0:q0 + 64, h // 2, :],
                          in_=_rep_row(rd[h:h + 1, :], 64))

        with tc.tile_pool(name="pa", bufs=6, space="PSUM") as pa_p, \
             tc.tile_pool(name="pz", bufs=2, space="PSUM") as pz_p:

            def norm_cc(cc):
                # pre-normalize qp by 1/D in place (bf16: 2x DVE throughput)
                nc.vector.tensor_mul(qpT[:, cc, :], qpT[:, cc, :],
                                     rbig[:, cc, :])

            def emit_attn_head(h):
                cc, q0 = h // 2, 64 * (h % 2)
                at = at0 if h % 2 == 0 else at1
                kv = kptv_sb[q0:q0 + 48, cc, 0:48]
                qh = qpT[q0:q0 + 48, cc, :].rearrange("p (r j) -> p j r", j=8)
                for jp in range(4):
                    pa = pa_p.tile([128, 512], F32, tag="pa")
                    nc.tensor.matmul(pa[0:48, :], kv, qh[:, 2 * jp, :],
                                     start=True, stop=True,
                                     tile_position=(q0, 0))
                    nc.tensor.matmul(pa[64:112, :], kv, qh[:, 2 * jp + 1, :],
                                     start=True, stop=True,
                                     tile_position=(q0, 64))
                    e0, e1 = ((nc.scalar, nc.vector) if jp % 2 == 0
                              else (nc.vector, nc.scalar))
                    _copy(e0, at[0:48, jp, :], pa[0:48, :])
                    _copy(e1, at[64:112, jp, :], pa[64:112, :])
                return at

            def emit_proj_head(h, at, split_last=False):
                for rc in range(4):
                    pz = pz_p.tile([128, C], F32)
                    for cc in range(4):
                        nc.tensor.matmul(pz[:], at[:, cc, 128 * rc:128 * (rc + 1)],
                                         wp_sb[:, cc, :],
                                         start=(cc == 0), stop=(cc == 3))
                    zo = zo_p.tile([128, C], F32)
                    r0 = 512 * h + 128 * rc
                    zeng = nc.vector if rc % 4 == 1 else nc.scalar
                    _copy(zeng, zo[:], pz[:])
                    nc.sync.dma_start(out=out[r0:r0 + 128, :], in_=zo[:])

            norm_cc(0)
            prev = emit_attn_head(0)
            for h in range(1, H):
                if h % 2 == 1 and h < H - 1:
                    norm_cc((h + 1) // 2)
                cur = emit_attn_head(h)
                emit_proj_head(h - 1, prev)
                prev = cur
            emit_proj_head(H - 1, prev, split_last=True)
    nc.finalize()
    return nc


def _prep_weights(Wqkv, Wproj, bproj):
    """Host-side weight prep: fold dn, pad head dims, build device layouts."""
    Wq = Wqkv[0:C, :]
    Wk = Wqkv[C:2 * C, :]
    Wv = Wqkv[2 * C:3 * C, :]
    wq = np.zeros((C, 512), np.float32)
    for h in range(H):
        wq[:, 64 * h:64 * h + 48] = (DN * Wq[48 * h:48 * (h + 1), :]).T
    wq = np.ascontiguousarray(
        wq.reshape(3, 128, 512)).astype(ml_dtypes.bfloat16)
    wkv = np.concatenate([(DN * Wk).T, Wv.T], axis=1).astype(np.float32)
    wkv = np.ascontiguousarray(
        wkv.reshape(3, 128, 768)).astype(ml_dtypes.bfloat16)
    # wp [128, 4, C]: row p<48 -> slot j=2*cc; row 64<=p<112 -> slot j=2*cc+1
    wp = np.zeros((128, 4, C), np.float32)
    WprojT = np.ascontiguousarray(Wproj.T)
    for cc in range(4):
        wp[0:48, cc, :] = WprojT[48 * (2 * cc):48 * (2 * cc) + 48, :]
        wp[64:112, cc, :] = WprojT[48 * (2 * cc + 1):48 * (2 * cc + 1) + 48, :]
    wp[127, 3, :] = bproj
    return (np.ascontiguousarray(wq.transpose(1, 0, 2)),
            np.ascontiguousarray(wkv.transpose(1, 0, 2)),
            wp.astype(ml_dtypes.bfloat16))


def _prep_x(xb):
    """x [N, C] f32 -> xT [128, 3, N] bf16 (c = kc*128 + p)."""
    xt = np.ascontiguousarray(
        xb.T.reshape(3, 128, N).transpose(1, 0, 2)).astype(ml_dtypes.bfloat16)
    return xt


def _run(inputs, trace=False):
    from concourse.bass_utils import run_bass_kernel_spmd

    x = np.asarray(inputs["x"], dtype=np.float32)
    Wqkv = np.asarray(inputs["Wqkv"], dtype=np.float32)
    Wproj = np.asarray(inputs["Wproj"], dtype=np.float32)
    bproj = np.asarray(inputs["bproj"], dtype=np.float32)
    wq, wkv, wp = _prep_weights(Wqkv, Wproj, bproj)

    if "nc" not in _NC_CACHE:
        _NC_CACHE["nc"] = build_nc()
    nc = _NC_CACHE["nc"]

    in_maps = [
        {"xt": _prep_x(x[b]), "wq": wq, "wkv": wkv, "wp": wp}
        for b in range(B)
    ]
    res = run_bass_kernel_spmd(nc, in_maps, list(range(8)), trace=trace)
    out = np.stack([res.results[b]["out"] for b in range(B)], axis=0)
    return out, res


def kernel(**inputs) -> np.ndarray:
    out, _ = _run(inputs, trace=False)
    return out


def kernel_profiled(**inputs):
    out, res = _run(inputs, trace=True)
    return out, res
